# revision 61
# baseline (speedup 1.0000x reference)
"""Trainium2 Bass kernel for causal multi-head attention with RoPE
(nn_Attention: S=2048, D=4096, H=32, hd=128), tensor-parallel over heads
across 8 NeuronCores.

Strategy (per core, 4 heads):
  - Q^T/K^T/V^T projections computed head-major directly in [hd, S] layout
    (lhsT = W tile [k,128], rhs = x^T tile [k, s-chunk]), bf16 matmuls.
  - RoPE applied in [hd, s] layout. Host permutes Wq/Wk columns per head to
    a [re(64); im(64)] split, so rotation = raw*C2 + swap(raw)*S2m where the
    half-swap is a 128x128 permutation matmul on the PE.
  - V^T is PE-transposed per 128-block into natural [t, hd] layout.
  - Scores computed directly TRANSPOSED per (head, t-tile 128): P^T block
    [t, s-chunk] = (K^T t-slice)^T Q^T, causally skipped. exp on ScalarE
    (no max subtraction - scores bounded); triangular mask applied
    multiplicatively on diagonal blocks; PV consumes P^T blocks directly
    (no PE transposes). Rowsums: DVE fp32 block-accumulate, then an
    all-ones f32r stationary matmul broadcasts sum-over-partitions to all
    128 PSUM partitions in one 512-cycle op; normalization folded into
    the A^T PSUM->SBUF copy as a per-column multiply by 1/rowsum.
  - Output projection O^T = Wo_loc^T A accumulated over the 4 local heads;
    each core writes a partial O^T [4096, 2048] fp32; host sums the 8
    partials and transposes (the row-sharded Wo all-reduce done on host).

Scheduling: emission order is the Tile priority order, so the kernel is
software-pipelined at emission level: each s-chunk's output projection is
interleaved with the next chunk's projection units; x^T strips prefetch
during attention; wo is persistent (loaded once); DMA issues split across
the two HWDGE queues (SP: x^T + outputs, ACT: weights + consts).
Measured on trn2: ~660 us/core, rel err 6.6e-3 vs fp32 reference.
"""

import math
import sys
import types

import numpy as np
import ml_dtypes

import concourse.bass as bass
import concourse.tile as tile
import concourse.mybir as mybir
from concourse import bass_utils

BF16 = mybir.dt.bfloat16
F16 = mybir.dt.float16
F32 = mybir.dt.float32
F32R = mybir.dt.float32r
P = 128


def enable_ldw_opt():
    """No-op: walrus --enable-ldw-opt=true rejects tile-emitted ldweights
    on this toolchain (CoreV3 'not compatible with LDW optimization'), and
    a fresh ldw-opt=false compile of the baseline reproduces its speed, so
    the flag is left at bass_utils's default (false)."""
    return


def install_ntff_hook_shim():
    """Make trace=True work under axon (antenv.axon_hooks is absent here)."""
    try:
        import antenv.axon_hooks  # noqa
        return
    except ImportError:
        pass
    try:
        import antenv
        from trn_agent_boot.trn_boot import _ntff_profile_via_ctypes
        hook = _ntff_profile_via_ctypes('/opt/axon/libaxon_pjrt.so')
        mod = types.ModuleType('antenv.axon_hooks')
        mod.get_axon_ntff_profile_hook = lambda: hook
        mod.set_axon_ntff_profile_hook = lambda h: None
        sys.modules['antenv.axon_hooks'] = mod
        antenv.axon_hooks = mod
    except Exception:
        pass


def dedup_ldweights(nc):
    """Remove an InstLdweights when the immediately preceding PE weight load
    has an identical stationary operand (consecutive matmuls sharing lhsT).
    Any waits on the removed load are transferred to the next instruction."""
    import concourse.mybir as _mb
    n = 0
    for f in nc.m.functions:
        for bb in f.blocks:
            new = []
            last_key = None
            pending_waits = []
            for inst in bb.instructions:
                ty = type(inst).__name__
                eng = getattr(inst, "engine", None)
                if eng == _mb.EngineType.PE:
                    if ty == "InstLdweights":
                        o = inst.ins[0]
                        key = (str(getattr(o, "memref", "")), o.offset,
                               str(o.ap), str(getattr(o, "dtype", "")),
                               getattr(inst, "is_transpose", None),
                               getattr(inst, "tile_position", None))
                        if key == last_key:
                            si = getattr(inst, "sync_info", None)
                            if si is not None and si.on_wait:
                                pending_waits.extend(si.on_wait)
                            n += 1
                            continue   # drop this load
                        last_key = key
                    elif ty in ("InstMatmult", "InstEventSemaphore", "InstNoOp"):
                        pass           # none of these clobber loaded weights
                    else:
                        last_key = None
                    if pending_waits:
                        si = getattr(inst, "sync_info", None)
                        if si is None:
                            inst.sync_info = _mb.SyncInfo(
                                on_wait=list(pending_waits), on_update=[])
                        else:
                            si.on_wait = list(pending_waits) + list(si.on_wait)
                        pending_waits = []
                new.append(inst)
            assert not pending_waits
            bb.instructions[:] = new
    return n


def split_excess_waits(nc, max_waits=1):
    """This walrus build accepts only one sync-wait per instruction; split
    extra waits into preceding wait-only NoOps on the same engine."""
    n = 0
    for f in nc.m.functions:
        for bb in f.blocks:
            new = []
            for inst in bb.instructions:
                si = getattr(inst, "sync_info", None)
                waits = list(si.on_wait) if (si is not None and si.on_wait) else []
                if len(waits) > max_waits:
                    extra, keep = waits[:-max_waits], waits[-max_waits:]
                    for j, w in enumerate(extra):
                        new.append(mybir.InstNoOp(
                            name=f"{inst.name}_sw{j}",
                            engine=inst.engine,
                            bass_nofuse=True,
                            sync_info=mybir.SyncInfo(on_wait=[w], on_update=[]),
                        ))
                    si.on_wait = keep
                    n += 1
                new.append(inst)
            bb.instructions[:] = new
    return n


class Cfg:
    def __init__(self, S=2048, D=4096, H_LOC=4, CHUNK=512, n_cores=8):
        self.S = S              # sequence length
        self.D = D              # model dim (= contraction dim of projections)
        self.H_LOC = H_LOC      # heads per core
        self.CHUNK = CHUNK      # s-chunk size (outer loop granularity)
        self.n_cores = n_cores
        self.NK = D // P        # k-tiles in projections
        self.NCH = S // CHUNK   # number of s-chunks
        self.TPC = CHUNK // P   # s/t tiles per chunk (must be 4 for 512)
        self.DLOC = H_LOC * P   # local head dims
        self.SCALE = 1.0 / math.sqrt(P)  # 1/sqrt(hd)


FULL = Cfg()


def build_program(cfg: Cfg):
    """Builds the per-core Bass/Tile program (SPMD: same NEFF on all cores)."""
    S, NK, H_LOC, CHUNK, NCH, TPC = cfg.S, cfg.NK, cfg.H_LOC, cfg.CHUNK, cfg.NCH, cfg.TPC

    nc = bass.Bass("TRN2", target_bir_lowering=False, debug=False,
                   num_devices=cfg.n_cores)

    # ---- DRAM I/O ----
    xt_d = nc.dram_tensor("xt", [NCH, 2, P, (NK // 2) * CHUNK], BF16,
                          kind="ExternalInput").ap()
    wq_d = nc.dram_tensor("wq", [H_LOC, P, NK * P], BF16, kind="ExternalInput").ap()
    wk_d = nc.dram_tensor("wk", [H_LOC, P, NK * P], BF16, kind="ExternalInput").ap()
    wv_d = nc.dram_tensor("wv", [H_LOC, P, NK * P], BF16, kind="ExternalInput").ap()
    wo_d = nc.dram_tensor("wo", [cfg.D // CHUNK, P, H_LOC * CHUNK], BF16,
                          kind="ExternalInput").ap()
    cos_d = nc.dram_tensor("cosS", [P, S], BF16, kind="ExternalInput").ap()
    sin_d = nc.dram_tensor("sinm", [P, S], BF16, kind="ExternalInput").ap()
    tri_d = nc.dram_tensor("tri", [P, P], BF16, kind="ExternalInput").ap()
    id_d = nc.dram_tensor("ident", [P, P], BF16, kind="ExternalInput").ap()
    swp_d = nc.dram_tensor("swap128", [P, P], BF16, kind="ExternalInput").ap()
    ones_d = nc.dram_tensor("ones128", [P, P], BF16, kind="ExternalInput").ap()
    ot_d = nc.dram_tensor("ot", [S, cfg.D], F16, kind="ExternalOutput").ap()

    from contextlib import ExitStack
    with tile.TileContext(nc) as tc, ExitStack() as es:
        if True:
            mkpool = lambda name, bufs, **kw: es.enter_context(
                tc.tile_pool(name=name, bufs=bufs, **kw))
            const_pool = mkpool("const", 1)
            persist = mkpool("persist", 1)
            xtp = mkpool("xtp", 2)
            wqkp = mkpool("wqk", 2)
            wop = mkpool("wop", 3)
            qtp = mkpool("qtp", H_LOC + 3)
            rawp = mkpool("rawp", 3)
            pp = mkpool("pp", 2 * (NCH * TPC))
            accp = mkpool("accp", 2)
            rsp = mkpool("rsp", 4)
            atp = mkpool("atp", 2 * H_LOC)
            osbp = mkpool("osbp", 4)
            onesp = mkpool("onesp", 1)
            psA = mkpool("psA", 2, space="PSUM")
            psS = mkpool("psS", 2, space="PSUM")
            psAT = mkpool("psAT", 2, space="PSUM")
            psPV = mkpool("psPV", 2, space="PSUM")

            # constants (emitted before the chunk loop but after nothing
            # critical; small transfers)
            tri = const_pool.tile([P, P], BF16, name="tri")
            nc.scalar.dma_start(tri, tri_d)
            ident = const_pool.tile([P, P], BF16, name="ident")
            nc.scalar.dma_start(ident, id_d)
            swap128 = const_pool.tile([P, P], BF16, name="swap128")
            nc.scalar.dma_start(swap128, swp_d)
            onesb = onesp.tile([P, P], BF16, name="onesb")
            nc.scalar.dma_start(onesb, ones_d)
            cosS = const_pool.tile([P, S], BF16, name="cosS")
            sinm = const_pool.tile([P, S], BF16, name="sinm")

            def emit_cos_sin(ch):
                # only chunk ch's 512 columns: keeps the startup-critical
                # scalar queue load small; later chunks' columns load during
                # their own (slack-rich) sections
                sl = slice(ch * CHUNK, (ch + 1) * CHUNK)
                nc.scalar.dma_start(cosS[:, sl], cos_d[:, sl])
                nc.scalar.dma_start(sinm[:, sl], sin_d[:, sl])

            # persistent K^T per head and natural V
            KT = []
            for h in range(H_LOC):
                kt_h = persist.tile([P, S], BF16, name=f"kt{h}", tag=f"kt{h}")
                KT.append(kt_h)
            Vn = persist.tile([P, S // P, H_LOC * P], BF16, name="vnat", tag="vnat")
            NGR = cfg.D // CHUNK
            wo_pers = persist.tile([P, NGR, H_LOC, CHUNK], BF16,
                                   name="wo_pers", tag="wo_pers")

            NKH = NK // 2
            xts_all = {}     # ch -> [half0, half1]
            qt_all = {}      # (ch, h) -> qt tile

            def alloc_xt(ch):
                xts = [xtp.tile([P, NKH, CHUNK], BF16,
                                name=f"xt_{ch}_{half}", tag="xt")
                       for half in range(2)]
                xts_all[ch] = xts

            def emit_xt_quarter(ch, q8, fine=False):
                """One of 8 quarter-DMAs for chunk ch's x^T strip."""
                half, q = divmod(q8, 4)
                xh = xts_all[ch][half]
                src = xt_d[ch, half].rearrange("p (k c) -> p k c", c=CHUNK)
                kq = NKH // 4
                if fine:
                    for j in range(kq):
                        ksl = slice(q * kq + j, q * kq + j + 1)
                        nc.sync.dma_start(xh[:, ksl, :], src[:, ksl, :])
                else:
                    ksl = slice(q * kq, (q + 1) * kq)
                    nc.sync.dma_start(xh[:, ksl, :], src[:, ksl, :])

            def emit_xt(ch):
                alloc_xt(ch)
                for q8 in range(8):
                    emit_xt_quarter(ch, q8, fine=(ch == 0 and q8 == 0))

            def emit_proj_unit(ch, which, h, after_w_hook=None, w_eng=None):
                """One projection unit: W load + 32 matmuls + epilogue."""
                s0 = ch * CHUNK
                xts = xts_all[ch]
                w_dram = {"q": wq_d, "k": wk_d, "v": wv_d}[which]
                wt = wqkp.tile([P, NK, P], BF16,
                               name=f"w{which}_{ch}_{h}", tag="wqk")
                wsrc = w_dram[h].rearrange("p (k m) -> p k m", m=P)
                npieces = 4 if (ch == 0 and h == 0) else 2
                eng = w_eng if w_eng is not None else nc.scalar
                for q in range(npieces):
                    ksl = slice(q * (NK // npieces), (q + 1) * (NK // npieces))
                    eng.dma_start(wt[:, ksl, :], wsrc[:, ksl, :])
                if after_w_hook is not None:
                    after_w_hook()
                ps = psA.tile([P, CHUNK], F32,
                              name=f"ps_{which}_{ch}_{h}", tag="psA")
                for k in range(NK):
                    nc.tensor.matmul(ps, wt[:, k, :],
                                     xts[k // NKH][:, k % NKH, :],
                                     start=(k == 0), stop=(k == NK - 1))
                raw = rawp.tile([P, CHUNK], BF16,
                                name=f"raw_{which}_{ch}_{h}", tag="raw")
                nc.any.tensor_copy(raw, ps)

                if which == "v":
                    # natural V per 128-block via PE transpose
                    pst = psA.tile([P, TPC, P], BF16,
                                   name=f"psvt_{ch}_{h}", tag="psA")
                    for tl in range(TPC):
                        nc.tensor.transpose(
                            pst[:, tl, :], raw[:, tl * P:(tl + 1) * P], ident)
                    nc.any.tensor_copy(
                        Vn[:, ch * TPC:(ch + 1) * TPC, h * P:(h + 1) * P], pst)
                else:
                    # RoPE: rot = raw*C2 + swap(raw)*S2m
                    ps2 = psA.tile([P, CHUNK], F32,
                                   name=f"psw_{which}_{ch}_{h}", tag="psA")
                    nc.tensor.matmul(ps2, swap128, raw, start=True, stop=True)
                    if which == "q":
                        dst = qtp.tile([P, CHUNK], BF16,
                                       name=f"qt_{ch}_{h}", tag="qt")
                        qt_all[(ch, h)] = dst
                    else:
                        dst = KT[h][:, s0:s0 + CHUNK]
                    tmp2 = rawp.tile([P, CHUNK], BF16,
                                     name=f"tmp2_{which}_{ch}_{h}", tag="tmp2")
                    nc.vector.tensor_mul(dst, raw, cosS[:, s0:s0 + CHUNK])
                    nc.vector.tensor_mul(tmp2, ps2, sinm[:, s0:s0 + CHUNK])
                    nc.vector.tensor_add(dst, dst, tmp2)

            emit_xt(0)
            first = [True]

            def _cos_hook():
                if first[0]:
                    # chunk-0 cos/sin emitted right after the first W load:
                    # they land during the first 32-matmul group, before the
                    # RoPE ops that consume them
                    emit_cos_sin(0)
                    first[0] = False

            # ---- merged per-chunk sections ----
            # section(ch) = units(ch) + scores/exp(ch) + PV(ch) + at(ch),
            # with O-proj(ch-1) groups as PE "filler" between everything:
            # ScalarE exps and the DVE rowsum chain always overlap dense
            # independent PE work, so the per-phase engine walls of the
            # two-phase layout disappear. O-proj(NCH-1) runs as a final
            # drain section.
            at_all = {}

            def emit_oproj_group(chp, stl, blk):
                """One O-proj group for chunk chp: s-tile stl x 2 ngs."""
                at_list = at_all[chp]
                ngs = [blk * 2, blk * 2 + 1]
                psos = [psS.tile([P, CHUNK], F32,
                                 name=f"pso_{chp}_{stl}_{blk}_{j}", tag="psS")
                        for j in range(2)]
                for h in range(H_LOC):
                    lhs = at_list[h][:, stl * P:(stl + 1) * P]
                    for j, ng in enumerate(ngs):
                        nc.tensor.matmul(psos[j], lhs, wo_pers[:, ng, h, :],
                                         start=(h == 0),
                                         stop=(h == H_LOC - 1))
                for j, ng in enumerate(ngs):
                    # fp16 partials: halves output DMA traffic; the 8
                    # per-core partials are summed in fp64 on the host so
                    # the added quantization is negligible
                    osb = osbp.tile([P, CHUNK], F16,
                                    name=f"osb_{chp}_{stl}_{blk}_{j}",
                                    tag="osb")
                    nc.any.tensor_copy(osb, psos[j])
                    srow = chp * CHUNK + stl * P
                    # last chunk's writes split across both queues: the
                    # scalar queue has no W loads left, and the tail drain
                    # halves
                    oq = nc.scalar if (chp == NCH - 1 and j % 2) else nc.sync
                    oq.dma_start(
                        ot_d[srow:srow + P, ng * CHUNK:(ng + 1) * CHUNK], osb)

            def make_fill(chp):
                if chp < 0:
                    return lambda n=1: None
                groups = iter([(stl, blk) for stl in range(TPC)
                               for blk in range(NGR // 2)])

                def fill(n=1):
                    for _ in range(n):
                        g = next(groups, None)
                        if g is None:
                            return
                        emit_oproj_group(chp, *g)
                return fill

            LAG = 4

            def emit_section(ch):
                n_tt = (ch + 1) * TPC
                fill = make_fill(ch - 1)
                at_cur = []
                state = {}

                def init_head(h):
                    state[h] = {
                        "ptbs": [],
                        "acc": accp.tile([P, CHUNK], F32,
                                         name=f"acc_{ch}_{h}", tag="acc"),
                        "psat": psPV.tile([P, CHUNK], F32,
                                          name=f"psat_{ch}_{h}", tag="psPV"),
                    }

                def emit_score_block(h, tb):
                    st = state[h]
                    s_lo = max(0, tb - ch * TPC)
                    pss = psAT.tile([P, CHUNK], F32,
                                    name=f"pss_{ch}_{h}_{tb}", tag="psAT")
                    nc.tensor.matmul(pss[:, s_lo * P:],
                                     KT[h][:, tb * P:(tb + 1) * P],
                                     qt_all[(ch, h)][:, s_lo * P:],
                                     start=True, stop=True)
                    ptb = pp.tile([P, CHUNK], BF16,
                                  name=f"p_{ch}_{h}_{tb}", tag="p")
                    nc.scalar.activation(ptb[:, s_lo * P:], pss[:, s_lo * P:],
                                         mybir.ActivationFunctionType.Exp,
                                         scale=cfg.SCALE)
                    if tb >= ch * TPC:
                        # diagonal 128-block: keep t <= s (upper-tri + diag)
                        d = tb - ch * TPC
                        nc.vector.tensor_mul(ptb[:, d * P:(d + 1) * P],
                                             ptb[:, d * P:(d + 1) * P], tri)
                    if tb == 0:
                        nc.vector.tensor_copy(st["acc"], ptb)
                    else:
                        nc.vector.tensor_add(st["acc"][:, s_lo * P:],
                                             st["acc"][:, s_lo * P:],
                                             ptb[:, s_lo * P:])
                    st["ptbs"].append((s_lo, ptb))

                def emit_pv_block(h, tb):
                    st = state[h]
                    s_lo, ptb = st["ptbs"][tb]
                    nc.tensor.matmul(st["psat"][:, s_lo * P:],
                                     Vn[:, tb, h * P:(h + 1) * P],
                                     ptb[:, s_lo * P:],
                                     start=(tb == 0), stop=(tb == n_tt - 1))

                def emit_rowsum(h):
                    # all-ones [128,128] bf16 stationary x acc -> every PSUM
                    # partition holds the sum-over-t (rowsum, broadcast)
                    st = state[h]
                    accb = rsp.tile([P, CHUNK], BF16,
                                    name=f"accb_{ch}_{h}", tag="accb")
                    nc.vector.tensor_copy(accb, st["acc"])
                    rs_ps = psPV.tile([P, CHUNK], F32,
                                      name=f"rsps_{ch}_{h}", tag="psPV")
                    nc.tensor.matmul(rs_ps, onesb, accb,
                                     start=True, stop=True)
                    st["rs_ps"] = rs_ps

                def emit_at(h):
                    # reciprocal + normalize in 128-col slices (pipelines the
                    # ~0.85us/slice DVE reciprocal with the at-muls)
                    st = state[h]
                    recipB = rsp.tile([P, CHUNK], F32,
                                      name=f"rcB_{ch}_{h}", tag="rcB")
                    at_h = atp.tile([P, CHUNK], BF16,
                                    name=f"at_{ch}_{h}", tag="at")
                    for sl in range(TPC):
                        c = slice(sl * P, (sl + 1) * P)
                        nc.vector.reciprocal(recipB[:, c], st["rs_ps"][:, c])
                        nc.vector.tensor_mul(at_h[:, c], st["psat"][:, c],
                                             recipB[:, c])
                    at_cur.append(at_h)

                for h in range(H_LOC):
                    order = ("k", "q", "v") if (ch == 0 and h == 0) \
                        else ("q", "k", "v")
                    for which in order:
                        # V-unit weights ride the SP queue (except chunk 0,
                        # where the SP queue is busy with the initial x^T
                        # strip): balances the two DMA queues
                        emit_proj_unit(ch, which, h,
                                       after_w_hook=_cos_hook if ch == 0
                                       else None,
                                       w_eng=nc.sync
                                       if (which == "v" and ch > 0) else None)
                        fill()
                    # rowsum/at for the previous head here: its DVE acc chain
                    # finished during this head's units -> no PE stall
                    if h > 0:
                        emit_rowsum(h - 1)
                        emit_at(h - 1)
                    init_head(h)
                    for i in range(n_tt + LAG):
                        if i < n_tt:
                            emit_score_block(h, i)
                        if i >= LAG:
                            emit_pv_block(h, i - LAG)
                        if i % 3 == 2:
                            fill()
                    if h == H_LOC - 1 and ch + 1 < NCH:
                        # prefetch next chunk's x^T strip (all units of this
                        # chunk have consumed theirs) and its cos/sin columns
                        emit_cos_sin(ch + 1)
                        alloc_xt(ch + 1)
                        for q8 in range(8):
                            emit_xt_quarter(ch + 1, q8)
                fill(2)
                emit_rowsum(H_LOC - 1)
                emit_at(H_LOC - 1)
                at_all[ch] = at_cur
                fill(99)

            for ch in range(NCH):
                emit_section(ch)
                if ch == 0:
                    # wo loaded once for the whole kernel; lands on the ACT
                    # queue behind chunk-0's W loads, in time for the first
                    # O-proj groups in section 1
                    for ng in range(NGR):
                        nc.scalar.dma_start(
                            wo_pers[:, ng],
                            wo_d[ng].rearrange("p (h c) -> p h c", c=CHUNK))
            # final O-proj drain for the last chunk
            final_fill = make_fill(NCH - 1)
            final_fill(99)

    dedup_ldweights(nc)
    split_excess_waits(nc)
    return nc


# ---------------- host-side data prep ----------------

def _tile_w(w_cols: np.ndarray, NK: int) -> np.ndarray:
    """[D, 128] per-head weight slice -> [128, NK*128] (k-part, k-outer*col)."""
    D = w_cols.shape[0]
    return np.ascontiguousarray(
        w_cols.reshape(NK, P, P).transpose(1, 0, 2).reshape(P, NK * P))


_ROPE_PERM = np.concatenate([np.arange(0, P, 2), np.arange(1, P, 2)])


def prepare_core_inputs(cfg: Cfg, core: int, x, wq, wk, wv, wo, cos, sin):
    """Builds the in_map (dict of numpy arrays) for one core."""
    bf = ml_dtypes.bfloat16
    S, D, H_LOC, CHUNK, NK, NCH = cfg.S, cfg.D, cfg.H_LOC, cfg.CHUNK, cfg.NK, cfg.NCH
    DLOC = cfg.DLOC
    c0 = core * DLOC

    out = {}
    # xt: [NCH, 2, 128, (NK//2)*CHUNK]
    xt = np.empty((NCH, 2, P, (NK // 2) * CHUNK), dtype=bf)
    xTb = x.T.astype(bf)  # [D, S]
    for ch in range(NCH):
        for half in range(2):
            blk = xTb[half * (D // 2):(half + 1) * (D // 2),
                      ch * CHUNK:(ch + 1) * CHUNK]          # [D/2, CHUNK]
            blk = blk.reshape(NK // 2, P, CHUNK).transpose(1, 0, 2)
            xt[ch, half] = blk.reshape(P, (NK // 2) * CHUNK)
    out["xt"] = xt

    for name, w, perm in (("wq", wq, True), ("wk", wk, True), ("wv", wv, False)):
        wt = np.empty((H_LOC, P, NK * P), dtype=bf)
        for h in range(H_LOC):
            cols = w[:, c0 + h * P: c0 + (h + 1) * P]
            if perm:
                cols = cols[:, _ROPE_PERM]
            wt[h] = _tile_w(cols.astype(bf), NK)
        out[name] = wt

    # wo: [D//CHUNK, 128, H_LOC*CHUNK]; wo[ng, p, h*CHUNK+nl] = Wo[c0+h*128+p, ng*CHUNK+nl]
    wo_loc = wo[c0:c0 + DLOC, :].astype(bf)  # [DLOC, D]
    wo_t = np.empty((D // CHUNK, P, H_LOC * CHUNK), dtype=bf)
    for ng in range(D // CHUNK):
        blk = wo_loc[:, ng * CHUNK:(ng + 1) * CHUNK]     # [DLOC, CHUNK]
        blk = blk.reshape(H_LOC, P, CHUNK).transpose(1, 0, 2)
        wo_t[ng] = blk.reshape(P, H_LOC * CHUNK)
    out["wo"] = wo_t

    cosT = cos.T.astype(np.float32)    # [64, S]
    sinT = sin.T.astype(np.float32)
    out["cosS"] = np.concatenate([cosT, cosT], 0).astype(bf)
    out["sinm"] = np.concatenate([-sinT, sinT], 0).astype(bf)

    # upper-triangular (incl diag): keep t <= s in [t, s]-layout diag blocks
    out["tri"] = np.triu(np.ones((P, P), np.float32)).astype(bf)
    out["ident"] = np.eye(P, dtype=np.float32).astype(bf)
    sw = np.zeros((P, P), np.float32)
    sw[(np.arange(P) + 64) % P, np.arange(P)] = 1.0
    out["swap128"] = sw.astype(bf)
    out["ones128"] = np.ones((P, P), np.float32).astype(bf)
    return out


_PROGRAM_CACHE = {}


def get_program(cfg: Cfg):
    key = (cfg.S, cfg.D, cfg.H_LOC, cfg.CHUNK, cfg.n_cores)
    if key not in _PROGRAM_CACHE:
        _PROGRAM_CACHE[key] = build_program(cfg)
    return _PROGRAM_CACHE[key]


def run(cfg: Cfg, inputs: dict, trace: bool = False):
    """Run the sharded kernel; returns (list of per-core ot partials, results obj)."""
    install_ntff_hook_shim()
    enable_ldw_opt()
    x = np.asarray(inputs["x"], np.float32)
    wq = np.asarray(inputs["weight_q"], np.float32)
    wk = np.asarray(inputs["weight_k"], np.float32)
    wv = np.asarray(inputs["weight_v"], np.float32)
    wo = np.asarray(inputs["weight_o"], np.float32)
    cos = np.asarray(inputs["freqs_cos"], np.float32)
    sin = np.asarray(inputs["freqs_sin"], np.float32)

    nc = get_program(cfg)
    in_maps = [prepare_core_inputs(cfg, c, x, wq, wk, wv, wo, cos, sin)
               for c in range(cfg.n_cores)]
    res = bass_utils.run_bass_kernel_spmd(
        nc, in_maps, core_ids=list(range(cfg.n_cores)), trace=trace)
    return [r["ot"] for r in res.results], res


def kernel(**inputs) -> np.ndarray:
    ots, _ = run(FULL, inputs, trace=False)
    acc = np.zeros_like(ots[0], dtype=np.float64)
    for ot in ots:
        acc += ot
    return np.ascontiguousarray(acc.astype(np.float32))

